# revision 1
# baseline (speedup 1.0000x reference)
"""ExplaiNN (dense_cnn) Trainium2 Bass kernel, 8-core SPMD.

Pipeline per reference:
  conv1d(4->300 units, K=19) + BN1 + exp + maxpool(7) -> per-unit fc1 (83->100)
  + BN2 + relu -> per-unit fc2 (100->1) + BN3 + relu -> final linear (300->2).

Distribution: conv+pool batch-sharded (16 b/core, all units), then an AllToAll
exchanges pooled features so fc1/fc2/final run unit-sharded (38 u/core, full
batch 128).  Final [128,2] partials are summed on host.

All BN affines are folded on host:
  y1 = a1*conv_raw + c1 ; pooled = exp(maxpool(y1))        (a1>0)
  fc1 psum = (a2*fc1_w)..pooled + c2  via ones-row         -> relu
  fc2 psum = (a3*fc2_w)..h2 + c3      via ones-row         -> relu
"""

import numpy as np
import ml_dtypes

B, N, L, K, C1 = 128, 300, 600, 19, 100
PS = 7
LC = 581          # conv outputs actually needed (l = 0..580; 83 pool windows)
LP = 83
NCLS = 2
EPS = 1e-5

NCORES = 8
BLOC = B // NCORES            # 16 batch per core in phase A
NPAD = 304                    # units padded to 8*38
ULOC = NPAD // NCORES         # 38 units per core in phase B
CK = 4 * K                    # 76 contraction rows for conv
UCHUNKS = [(0, 128), (128, 128), (256, 48)]   # (start, real size) unit chunks
WCONV_COLS = 384          # conv weight cols padded so every matmul is M=128
# fp32r matmuls need even free size: two overlapping 294-wide chunks.
# (l0, ncols, q0, nwin): window q41 is computed twice, identically.
NSPLIT = [(0, 294, 0, 42), (287, 294, 41, 42)]

_CACHE = {}


def _build_bass():
    import concourse.bass as bass
    import concourse.bacc as bacc
    import concourse.mybir as mybir
    import concourse.tile as tile

    f32, bf16, f32r = mybir.dt.float32, mybir.dt.bfloat16, mybir.dt.float32r

    # Bacc (not plain Bass): its finalize() runs the wait-splitting passes
    # (move_matmul_waits_to_ldweights / generate_event_semaphores) that keep
    # every TPB command within its single hardware sync-wait slot.
    nc = bacc.Bacc("TRN2")
    xloc = nc.declare_dram_parameter("xloc", [BLOC, 4, L], f32r, isOutput=False)
    wconv = nc.declare_dram_parameter("wconv", [CK, WCONV_COLS], f32r, isOutput=False)
    a1 = nc.declare_dram_parameter("a1", [128, 3], f32, isOutput=False)
    c1 = nc.declare_dram_parameter("c1", [128, 3], f32, isOutput=False)
    w1aug = nc.declare_dram_parameter("w1aug", [LP + 1, ULOC * C1], bf16, isOutput=False)
    w2aug = nc.declare_dram_parameter("w2aug", [C1 + 1, ULOC], bf16, isOutput=False)
    fwrep = nc.declare_dram_parameter("fwrep", [128, NCLS, ULOC], bf16, isOutput=False)
    out_part = nc.declare_dram_parameter("out_part", [B, NCLS], f32, isOutput=True)

    with tile.TileContext(nc) as tc:
        with (
            tc.tile_pool(name="dram", bufs=1, space="DRAM") as dram_pool,
            tc.tile_pool(name="singles", bufs=1) as singles,
            tc.tile_pool(name="im2col", bufs=BLOC) as im2col_pool,
            tc.tile_pool(name="praw", bufs=1) as praw_pool,
            tc.tile_pool(name="pexp", bufs=1) as pexp_pool,
            tc.tile_pool(name="scratch", bufs=1, space="PSUM") as scratch_pool,
        ):
            # DRAM exchange buffers
            # 84 p-rows: 0..82 pooled features, row 83 = ones (fc1 bias row,
            # produced by transposing the 1.0-memset pad columns of pexp)
            p2p_in = dram_pool.tile([NCORES, LP + 1, BLOC, ULOC], bf16,
                                    name="p2p_in")
            p2p_out = dram_pool.tile([NCORES, LP + 1, BLOC, ULOC], bf16,
                                     name="p2p_out")

            wconv_sb = singles.tile([CK, WCONV_COLS], f32r)
            nc.sync.dma_start(out=wconv_sb, in_=wconv[:, :])
            a1_sb = singles.tile([128, 3], f32)
            nc.sync.dma_start(out=a1_sb, in_=a1[:, :])
            c1_sb = singles.tile([128, 3], f32)
            nc.sync.dma_start(out=c1_sb, in_=c1[:, :])

            # im2col: one [76, 581] f32 tile per local batch element.
            # row (c*19+k), col l  <-  x[b, c, l+k]
            im2b = []
            for b in range(BLOC):
                t = im2col_pool.tile([CK, LC], f32r, name=f"im2_{b}", tag="im2col")
                src = bass.AP(
                    tensor=xloc,
                    offset=b * 4 * L,
                    ap=[[L, 4], [1, K], [1, LC]],
                )
                nc.sync.dma_start(out=t, in_=src)
                im2b.append(t)

            praw = []
            pexp = []
            for ci, (u0, P) in enumerate(UCHUNKS):
                praw.append(praw_pool.tile([128, BLOC, LP], f32,
                                           name=f"praw{ci}"))
                pexp.append(pexp_pool.tile([128, BLOC, 128], bf16,
                                           name=f"pexp{ci}"))
                # pad cols 83..127 with 1.0: DMA-transpose reads full 128-wide
                # rows, and transposed row 83 becomes the fc1 bias ones-row
                nc.gpsimd.memset(pexp[ci][:, :, LP:128], 1.0)

            # PE matmuls (notably the fp32r LDW path) only accept one sync
            # wait; a dummy bf16 matmul reading a freshly-DMA'd tile absorbs
            # its semaphore so the first real matmul of a phase needs one.
            def absorb(tile_ap):
                # tile_ap: a [1, >=2]-elem slice of a freshly-DMA'd tile
                s = scratch_pool.tile([2, 2], f32, name="dummy", tag="dummy")
                src = tile_ap.bitcast(bf16) if tile_ap.dtype != bf16 else tile_ap
                src = src[0:1, 0:2]
                nc.tensor.matmul(out=s, lhsT=src, rhs=src,
                                 start=True, stop=True)

            # ---------------- conv + maxpool ----------------
            with tc.tile_pool(name="psA", bufs=4, space="PSUM") as psum_pool:
                absorb(wconv_sb[0:1, 0:2])
                for ci, (u0, P) in enumerate(UCHUNKS):
                    lhsT = wconv_sb[:, u0:u0 + 128]   # M=128 (fp32r needs it)
                    for b in range(BLOC):
                        for (l0, ncol, q0, nwin) in NSPLIT:
                            ps = psum_pool.tile([128, 294], f32, name="ps", tag="ps")
                            nc.tensor.matmul(
                                out=ps[:, 0:ncol],
                                lhsT=lhsT,
                                rhs=im2b[b][:, l0:l0 + ncol],
                                start=True, stop=True,
                            )
                            nc.vector.reduce_max(
                                out=praw[ci][0:P, b, q0:q0 + nwin],
                                in_=ps[0:P, 0:ncol].rearrange(
                                    "p (q w) -> p q w", w=PS),
                                axis=mybir.AxisListType.X,
                            )

            # BN1+exp (pool commutes with monotone exp), then transpose
            # to [p, b, u] and stage the exchange payload
            pTall = singles.tile([LP + 1, NCORES, BLOC, ULOC], bf16)
            poolT = singles.tile([128, BLOC, NPAD], bf16)
            for ci, (u0, P) in enumerate(UCHUNKS):
                nc.scalar.activation(
                    out=pexp[ci][0:P, :, 0:LP],
                    in_=praw[ci][0:P, :, :],
                    func=mybir.ActivationFunctionType.Exp,
                    scale=a1_sb[0:P, ci:ci + 1],
                    bias=c1_sb[0:P, ci:ci + 1],
                )
                for b in range(BLOC):
                    nc.sync.dma_start(
                        out=poolT[:, b, u0:u0 + P],
                        in_=pexp[ci][0:P, b, :],
                        transpose=True,
                    )
            for j in range(NCORES):
                nc.sync.dma_start(
                    out=p2p_in[j, :, :, :],
                    in_=poolT[0:LP + 1, :, j * ULOC:(j + 1) * ULOC],
                )
            nc.gpsimd.collective_compute(
                "AllToAll",
                mybir.AluOpType.bypass,
                replica_groups=[list(range(NCORES))],
                ins=[p2p_in[:]],
                outs=[p2p_out[:]],
            )
            # received: [84, (core, b, uloc)] into pTall
            src = bass.AP(
                tensor=p2p_out.tensor,
                offset=0,
                ap=[[BLOC * ULOC, LP + 1], [(LP + 1) * BLOC * ULOC, NCORES],
                    [ULOC, BLOC], [1, ULOC]],
            )
            nc.sync.dma_start(out=pTall[:, :, :, :], in_=src)

            # ---------------- fc1 ----------------
            w1_sb = singles.tile([LP + 1, ULOC * C1], bf16)
            nc.sync.dma_start(out=w1_sb, in_=w1aug[:, :])
            w2_sb = singles.tile([C1 + 1, ULOC], bf16)
            nc.sync.dma_start(out=w2_sb, in_=w2aug[:, :])
            fw_sb = singles.tile([128, NCLS, ULOC], bf16)
            nc.sync.dma_start(out=fw_sb, in_=fwrep[:, :, :])

            h2_sb = singles.tile([128, ULOC * B], bf16)
            # row 100 = fc2 bias ones row; on gpsimd to keep DVE free
            nc.gpsimd.memset(h2_sb[96:128, :], 1.0)

            with tc.tile_pool(name="psB", bufs=3, space="PSUM") as psum_b:
                absorb(w1_sb[0:1, 0:2])
                ngroups = (ULOC + 3) // 4
                for g in range(ngroups):
                    un = min(4, ULOC - 4 * g)
                    psf = psum_b.tile([C1, 512], f32, name="psf", tag="psf")
                    for k in range(un):
                        u = 4 * g + k
                        # rhs [84 part, (core, b)] = pooled cols for unit u
                        rhs = pTall[:, :, :, u]
                        nc.tensor.matmul(
                            out=psf[:, k * B:(k + 1) * B],
                            lhsT=w1_sb[:, u * C1:(u + 1) * C1],
                            rhs=rhs,
                            start=True, stop=True,
                        )
                    dst = h2_sb[0:C1, 4 * g * B:(4 * g + un) * B]
                    if g % 2 == 0:
                        nc.scalar.activation(
                            out=dst, in_=psf[:, 0:un * B],
                            func=mybir.ActivationFunctionType.Relu,
                        )
                    else:
                        nc.vector.tensor_scalar_max(
                            out=dst, in0=psf[:, 0:un * B], scalar1=0.0,
                        )

                # ---------------- fc2 ----------------
                absorb(w2_sb[0:1, 0:2])
                ps38 = psum_b.tile([B, ULOC], f32, name="ps38", tag="ps38",
                                   bufs=1)
                for u in range(ULOC):
                    nc.tensor.matmul(
                        out=ps38[:, u:u + 1],
                        lhsT=h2_sb[0:C1 + 1, u * B:(u + 1) * B],
                        rhs=w2_sb[:, u:u + 1],
                        start=True, stop=True,
                    )
                h3_sb = singles.tile([B, ULOC], bf16)
                nc.scalar.activation(
                    out=h3_sb, in_=ps38,
                    func=mybir.ActivationFunctionType.Relu,
                )

            # ---------------- final linear (partial over my units) ---------
            out_sb = singles.tile([B, NCLS], f32)
            prod = singles.tile([B, ULOC], f32)
            for cls in range(NCLS):
                nc.vector.tensor_mul(out=prod, in0=h3_sb, in1=fw_sb[:, cls, :])
                nc.vector.reduce_sum(
                    out=out_sb[:, cls:cls + 1], in_=prod,
                    axis=mybir.AxisListType.X,
                )
            nc.sync.dma_start(out=out_part[:, :], in_=out_sb)

    # Bacc defers register allocation etc. to finalize(); run_bass_via_pjrt
    # binds the module as-is, so finalize here.
    nc.finalize()
    return nc


def _host_prep(inputs):
    """Fold BN affines, pad units to 304, build per-core input maps."""
    x = np.asarray(inputs["x"], np.float32)
    conv_w = np.asarray(inputs["conv_w"], np.float32)
    conv_b = np.asarray(inputs["conv_b"], np.float32)
    g1, b1 = np.asarray(inputs["bn1_g"], np.float32), np.asarray(inputs["bn1_b"], np.float32)
    m1, v1 = np.asarray(inputs["bn1_m"], np.float32), np.asarray(inputs["bn1_v"], np.float32)
    fc1_w, fc1_b = np.asarray(inputs["fc1_w"], np.float32), np.asarray(inputs["fc1_b"], np.float32)
    g2, b2 = np.asarray(inputs["bn2_g"], np.float32), np.asarray(inputs["bn2_b"], np.float32)
    m2, v2 = np.asarray(inputs["bn2_m"], np.float32), np.asarray(inputs["bn2_v"], np.float32)
    fc2_w, fc2_b = np.asarray(inputs["fc2_w"], np.float32), np.asarray(inputs["fc2_b"], np.float32)
    g3, b3 = np.asarray(inputs["bn3_g"], np.float32), np.asarray(inputs["bn3_b"], np.float32)
    m3, v3 = np.asarray(inputs["bn3_m"], np.float32), np.asarray(inputs["bn3_v"], np.float32)
    final_w = np.asarray(inputs["final_w"], np.float32)
    final_b = np.asarray(inputs["final_b"], np.float32)

    a1 = g1 / np.sqrt(v1 + EPS)                      # [300] > 0
    c1 = a1 * (conv_b - m1) + b1                     # [300]
    a2 = g2 / np.sqrt(v2 + EPS)                      # [300,100]
    c2 = b2 - a2 * m2 + a2 * fc1_b                   # [300,100]
    a3 = g3 / np.sqrt(v3 + EPS)                      # [300]
    c3 = a3 * (fc2_b - m3) + b3                      # [300]

    # conv weights [76, 384]; im2col row = c*19+k; cols ≥300 are zero pad
    wconv = np.zeros((CK, WCONV_COLS), np.float32)
    wconv[:, :N] = conv_w.transpose(1, 2, 0).reshape(CK, N)

    a1p = np.ones(NPAD, np.float32)
    c1p = np.zeros(NPAD, np.float32)
    a1p[:N], c1p[:N] = a1, c1
    a1t = np.ones((128, 3), np.float32)
    c1t = np.zeros((128, 3), np.float32)
    for ci, (u0, P) in enumerate(UCHUNKS):
        a1t[0:P, ci] = a1p[u0:u0 + P]
        c1t[0:P, ci] = c1p[u0:u0 + P]

    # fc1: lhsT [84, 100] per unit; rows 0..82 = a2*w1 (p-major),
    # row 83 = c2 (pairs with the ones row of pTall)
    w1aug = np.zeros((NPAD, LP + 1, C1), np.float32)
    w1aug[:N, :LP, :] = (fc1_w * a2[:, :, None]).transpose(0, 2, 1)
    w1aug[:N, LP, :] = c2

    # fc2: rhs [101, 1] per unit; rows 0..99 = a3*w2, row 100 = c3
    w2aug = np.zeros((NPAD, C1 + 1), np.float32)
    w2aug[:N, :C1] = fc2_w * a3[:, None]
    w2aug[:N, C1] = c3

    fwpad = np.zeros((NCLS, NPAD), np.float32)
    fwpad[:, :N] = final_w

    bf = ml_dtypes.bfloat16
    in_maps = []
    for i in range(NCORES):
        us = slice(i * ULOC, (i + 1) * ULOC)
        w1c = w1aug[us].transpose(1, 0, 2).reshape(LP + 1, ULOC * C1)
        w2c = w2aug[us].T                                   # [101, 38]
        fwc = np.broadcast_to(fwpad[:, us], (128, NCLS, ULOC))
        in_maps.append({
            "xloc": np.ascontiguousarray(x[i * BLOC:(i + 1) * BLOC]),
            "wconv": wconv,
            "a1": a1t,
            "c1": c1t,
            "w1aug": np.ascontiguousarray(w1c).astype(bf),
            "w2aug": np.ascontiguousarray(w2c).astype(bf),
            "fwrep": np.ascontiguousarray(fwc).astype(bf),
        })
    return in_maps, final_b


def kernel(**inputs):
    from concourse.bass_utils import run_bass_kernel_spmd

    if "nc" not in _CACHE:
        _CACHE["nc"] = _build_bass()
    nc = _CACHE["nc"]

    in_maps, final_b = _host_prep(inputs)
    res = run_bass_kernel_spmd(nc, in_maps, core_ids=list(range(NCORES)))
    out = np.zeros((B, NCLS), np.float32)
    for r in res.results:
        out += r["out_part"]
    out += final_b[None, :]
    return out



# revision 17
# speedup vs baseline: 1.3205x; 1.3205x over previous
"""ExplaiNN (dense_cnn) Trainium2 Bass kernel, 8-core SPMD. v2.

Pipeline per reference:
  conv1d(4->300 units, K=19) + BN1 + exp + maxpool(7) -> per-unit fc1 (83->100)
  + BN2 + relu -> per-unit fc2 (100->1) + BN3 + relu -> final linear (300->2).

Distribution: conv+pool batch-sharded (16 b/core, all units), then an AllToAll
exchanges pooled features so fc1/fc2/final run unit-sharded (38 u/core, full
batch 128).  Final [128,2] partials are summed on host.

v2 vs v1:
  - conv in bf16 (fp32r was ~4x slower + no FWL on LDWEIGHTS)
  - BN1 affine folded into conv weights (a1*w) + a ones-row carrying c1,
    so psum is already normalized; exp applied post-pool with no scale/bias
  - maxpool split across engines: DVE reduce_max direct from PSUM for some
    batches, DVE/ACT copy to SBUF + GpSimd pairwise-max tree for the rest
  - pexp -> poolT transpose via pipelined PE transposes (identity matmul)
    + batched DVE evacuation, replacing 48 serial DMA_TRANSPOSEs (60us of
    HWDGE sequencer occupancy)
  - fc1 weights padded 100->128 h cols so LDWEIGHTS gets FWL
"""

import numpy as np
import ml_dtypes

B, N, L, K, C1 = 128, 300, 600, 19, 100
PS = 7
LP = 83            # pool windows
LC2 = 582          # psum conv cols (581 needed, +1 garbage for even splits)
NCLS = 2
EPS = 1e-5

NCORES = 8
BLOC = B // NCORES            # 16 batch per core in phase A
NPAD = 304                    # units padded to 8*38
ULOC = NPAD // NCORES         # 38 units per core in phase B
CK = 76                       # 4*19 contraction rows
WCONV_COLS = 384              # conv weight cols padded so every matmul is M=128
QP = 96                       # pexp q-cols padded (83 pools + ones col at 83)
HPAD = 128                    # fc1 h padded 100->128 (FWL needs 128 weight cols)

# conv matmul column splits: even sizes, none crossing the 512-col (2KB)
# PSUM bank boundary of the 2-bank [128, 582] fp32 tile
CSPLIT = [(0, 294), (294, 218), (512, 70)]

# per-batch pool mode, same for each chunk: 'd'=DVE reduce_max direct,
# 'a'=ACT copy to SBUF (w-major) + DVE bf16 pairwise-max tree (2x mode)
POOL_MODES = "dddddddd" + "aaaaaaaa"

_CACHE = {}


def _build_bass():
    import concourse.bass as bass
    import concourse.bacc as bacc
    import concourse.mybir as mybir
    import concourse.tile as tile

    f32, bf16 = mybir.dt.float32, mybir.dt.bfloat16

    nc = bacc.Bacc("TRN2")
    xloc = nc.declare_dram_parameter("xloc", [BLOC, 4, L], bf16, isOutput=False)
    wconv = nc.declare_dram_parameter("wconv", [CK, WCONV_COLS], bf16, isOutput=False)
    w1aug = nc.declare_dram_parameter("w1aug", [LP + 1, ULOC * HPAD], bf16, isOutput=False)
    w2aug = nc.declare_dram_parameter("w2aug", [C1 + 1, ULOC], bf16, isOutput=False)
    fwrep = nc.declare_dram_parameter("fwrep", [128, NCLS, ULOC], bf16, isOutput=False)
    ident = nc.declare_dram_parameter("ident", [128, 128], bf16, isOutput=False)
    onesrow = nc.declare_dram_parameter("onesrow", [1, ULOC * B], bf16, isOutput=False)
    c1p = nc.declare_dram_parameter("c1p", [128, 3], mybir.dt.float32, isOutput=False)
    out_part = nc.declare_dram_parameter("out_part", [B, NCLS], f32, isOutput=True)

    n_copy = sum(1 for m in POOL_MODES if m != 'd')   # copy-path slots per chunk
    b_copy0 = POOL_MODES.index('a')

    with tile.TileContext(nc) as tc:
        with (
            tc.tile_pool(name="dram", bufs=1, space="DRAM") as dram_pool,
            tc.tile_pool(name="singles", bufs=1) as singles,
            tc.tile_pool(name="im2col", bufs=BLOC) as im2col_pool,
            tc.tile_pool(name="praw", bufs=1) as praw_pool,
            tc.tile_pool(name="praws", bufs=2) as praws_pool,
            tc.tile_pool(name="gpst", bufs=2) as gpst_pool,
            tc.tile_pool(name="pexp", bufs=1) as pexp_pool,
            tc.tile_pool(name="scratch", bufs=1, space="PSUM") as scratch_pool,
        ):
            # DRAM exchange buffers: [dest core, p-row, batch, unit]
            p2p_in = dram_pool.tile([NCORES, LP + 1, BLOC, ULOC], bf16,
                                    name="p2p_in")
            p2p_out = dram_pool.tile([NCORES, LP + 1, BLOC, ULOC], bf16,
                                     name="p2p_out")

            wconv_sb = singles.tile([CK, WCONV_COLS], bf16)
            nc.sync.dma_start(out=wconv_sb, in_=wconv[:, :])
            ident_sb = singles.tile([128, 128], bf16)
            nc.sync.dma_start(out=ident_sb, in_=ident[:, :])
            c1_sb = singles.tile([128, 3], f32)
            nc.sync.dma_start(out=c1_sb, in_=c1p[:, :])

            # im2col: [77, 582] bf16 per local batch element.
            # rows 0..75: row (c*19+k), col l  <-  x[b, c, l+k]  (l+k <= 599)
            # row 76: ones (pairs with wconv row 76 = c1 -> psum = a1*conv+c1)
            im2b = []
            for b in range(BLOC):
                t = im2col_pool.tile([CK, LC2], bf16, name=f"im2_{b}", tag="im2col")
                src = bass.AP(
                    tensor=xloc,
                    offset=b * 4 * L,
                    ap=[[L, 4], [1, K], [1, LC2]],
                )
                nc.sync.dma_start(out=t[:, :], in_=src)
                im2b.append(t)

            praw = []       # pooled, BN1-normalized, pre-exp [128, 16, 83]
            praw_s = []     # raw conv rows staged for the GpSimd tree
            pexp = []       # exp'd pooled features [128, 16, 96], col 83 = ones
            for ci in range(3):
                praw.append(praw_pool.tile([128, BLOC, LP], bf16, name=f"praw{ci}"))
                praw_s.append(praws_pool.tile([128, n_copy, PS * LP], bf16,
                                              name=f"praws", tag="praws"))
                p = pexp_pool.tile([128, BLOC, QP], bf16, name=f"pexp{ci}")
                nc.gpsimd.memset(p[:, :, LP:QP], 1.0)
                pexp.append(p)

            def absorb(tile_ap):
                s = scratch_pool.tile([2, 2], f32, name="dummy", tag="dummy")
                src = tile_ap.bitcast(bf16) if tile_ap.dtype != bf16 else tile_ap
                src = src[0:1, 0:2]
                nc.tensor.matmul(out=s, lhsT=src, rhs=src,
                                 start=True, stop=True)

            # ---------------- conv + pool dispatch ----------------
            with tc.tile_pool(name="psA", bufs=3, space="PSUM") as psum_pool:
                absorb(wconv_sb[0:1, 0:2])
                for ci in range(3):
                    u0 = 128 * ci
                    lhsT = wconv_sb[:, u0:u0 + 128]
                    slot = 0
                    for b in range(BLOC):
                        ps = psum_pool.tile([128, LC2], f32, name="ps", tag="ps")
                        for (l0, ncol) in CSPLIT:
                            nc.tensor.matmul(
                                out=ps[:, l0:l0 + ncol],
                                lhsT=lhsT,
                                rhs=im2b[b][:, l0:l0 + ncol],
                                start=True, stop=True,
                            )
                        mode = POOL_MODES[b]
                        if mode == 'd':
                            nc.vector.reduce_max(
                                out=praw[ci][:, b, :],
                                in_=ps[:, 0:581].rearrange("p (q w) -> p q w", w=PS),
                                axis=mybir.AxisListType.X,
                            )
                        else:
                            # copy in w-major order so the DVE tree below is
                            # contiguous (bf16 2x mode): col w*83+q <- psum 7q+w
                            nc.scalar.copy(
                                out=praw_s[ci][:, slot, :].rearrange(
                                    "p (w q) -> p w q", q=LP),
                                in_=ps[:, 0:581].rearrange("p (q w) -> p w q", w=PS),
                            )
                            slot += 1

                    # DVE bf16 pairwise-max tree over the copy-path batches
                    if n_copy:
                        s = praw_s[ci]
                        w_of = lambda w: s[:, :, w * LP:(w + 1) * LP]
                        tA = gpst_pool.tile([128, n_copy, LP], bf16, name="tA", tag="tA")
                        tB = gpst_pool.tile([128, n_copy, LP], bf16, name="tB", tag="tB")
                        tC = gpst_pool.tile([128, n_copy, LP], bf16, name="tC", tag="tC")
                        tD = gpst_pool.tile([128, n_copy, LP], bf16, name="tD", tag="tD")
                        tE = gpst_pool.tile([128, n_copy, LP], bf16, name="tE", tag="tE")
                        nc.vector.tensor_max(out=tA, in0=w_of(0), in1=w_of(1))
                        nc.vector.tensor_max(out=tB, in0=w_of(2), in1=w_of(3))
                        nc.vector.tensor_max(out=tC, in0=w_of(4), in1=w_of(5))
                        nc.vector.tensor_max(out=tD, in0=tA, in1=tB)
                        nc.vector.tensor_max(out=tE, in0=tC, in1=w_of(6))
                        nc.vector.tensor_max(
                            out=praw[ci][:, b_copy0:b_copy0 + n_copy, :],
                            in0=tD, in1=tE)

                    # exp over the chunk's pooled features (normalized already)
                    nc.scalar.activation(
                        out=pexp[ci][:, :, 0:LP],
                        in_=praw[ci][:, :, :],
                        func=mybir.ActivationFunctionType.Exp,
                        bias=c1_sb[:, ci:ci + 1],
                    )

            # ---------------- transpose via PE + evac ----------------
            # poolT[p, b, u] = pexp[u, b, p]; p-row 83 = ones (fc1 bias row)
            poolT = singles.tile([LP + 1, BLOC, NPAD], bf16)
            with tc.tile_pool(name="psT", bufs=3, space="PSUM") as psumt_pool:
                absorb(ident_sb[0:1, 0:2])
                for ci in range(3):
                    u0 = 128 * ci
                    un = min(128, NPAD - u0)       # 128,128,48
                    for b0 in range(0, BLOC, 4):
                        pst = psumt_pool.tile([QP, 4, 128], bf16, name="psT", tag="psT")
                        for k in range(4):
                            nc.tensor.transpose(
                                out=pst[:, k, :],
                                in_=pexp[ci][:, b0 + k, :],
                                identity=ident_sb[:, :],
                            )
                        nc.vector.tensor_copy(
                            out=poolT[0:LP + 1, b0:b0 + 4, u0:u0 + un],
                            in_=pst[0:LP + 1, :, 0:un],
                        )

            # ---------------- exchange ----------------
            for j in range(NCORES):
                nc.sync.dma_start(
                    out=p2p_in[j, :, :, :],
                    in_=poolT[0:LP + 1, :, j * ULOC:(j + 1) * ULOC],
                )
            nc.gpsimd.collective_compute(
                "AllToAll",
                mybir.AluOpType.bypass,
                replica_groups=[list(range(NCORES))],
                ins=[p2p_in[:]],
                outs=[p2p_out[:]],
            )
            # received: [84, (core, b, uloc)] into pTall
            pTall = singles.tile([LP + 1, NCORES, BLOC, ULOC], bf16)
            src = bass.AP(
                tensor=p2p_out.tensor,
                offset=0,
                ap=[[BLOC * ULOC, LP + 1], [(LP + 1) * BLOC * ULOC, NCORES],
                    [ULOC, BLOC], [1, ULOC]],
            )
            nc.sync.dma_start(out=pTall[:, :, :, :], in_=src)

            # ---------------- fc1 ----------------
            w1_sb = singles.tile([LP + 1, ULOC * HPAD], bf16)
            nc.sync.dma_start(out=w1_sb, in_=w1aug[:, :])
            w2_sb = singles.tile([C1 + 1, ULOC], bf16)
            nc.sync.dma_start(out=w2_sb, in_=w2aug[:, :])
            fw_sb = singles.tile([128, NCLS, ULOC], bf16)
            nc.sync.dma_start(out=fw_sb, in_=fwrep[:, :, :])

            h2_sb = singles.tile([C1 + 1, ULOC * B], bf16)
            nc.sync.dma_start(out=h2_sb[C1:C1 + 1, :], in_=onesrow[:, :])

            with tc.tile_pool(name="psB", bufs=3, space="PSUM") as psum_b:
                absorb(w1_sb[0:1, 0:2])
                ngroups = (ULOC + 3) // 4
                for g in range(ngroups):
                    un = min(4, ULOC - 4 * g)
                    psf = psum_b.tile([HPAD, 4, B], f32, name="psf", tag="psf")
                    for k in range(un):
                        u = 4 * g + k
                        nc.tensor.matmul(
                            out=psf[:, k, :],
                            lhsT=w1_sb[:, u * HPAD:(u + 1) * HPAD],
                            rhs=pTall[:, :, :, u],
                            start=True, stop=True,
                        )
                    nc.vector.tensor_scalar_max(
                        out=h2_sb[0:C1, 4 * g * B:(4 * g + un) * B],
                        in0=psf[0:C1, 0:un, :],
                        scalar1=0.0,
                    )

                # ---------------- fc2 ----------------
                absorb(w2_sb[0:1, 0:2])
                ps38 = psum_b.tile([B, ULOC], f32, name="ps38", tag="ps38",
                                   bufs=1)
                for u in range(ULOC):
                    nc.tensor.matmul(
                        out=ps38[:, u:u + 1],
                        lhsT=h2_sb[0:C1 + 1, u * B:(u + 1) * B],
                        rhs=w2_sb[:, u:u + 1],
                        start=True, stop=True,
                    )
                h3_sb = singles.tile([B, ULOC], bf16)
                nc.vector.tensor_scalar_max(out=h3_sb, in0=ps38, scalar1=0.0)

            # ---------------- final linear (partial over my units) ---------
            out_sb = singles.tile([B, NCLS], f32)
            prod = singles.tile([B, ULOC], f32)
            for cls in range(NCLS):
                nc.vector.tensor_mul(out=prod, in0=h3_sb, in1=fw_sb[:, cls, :])
                nc.vector.reduce_sum(
                    out=out_sb[:, cls:cls + 1], in_=prod,
                    axis=mybir.AxisListType.X,
                )
            nc.sync.dma_start(out=out_part[:, :], in_=out_sb)

    nc.finalize()
    return nc


def _host_prep(inputs):
    """Fold BN affines, pad units to 304, build per-core input maps."""
    x = np.asarray(inputs["x"], np.float32)
    conv_w = np.asarray(inputs["conv_w"], np.float32)
    conv_b = np.asarray(inputs["conv_b"], np.float32)
    g1, b1 = np.asarray(inputs["bn1_g"], np.float32), np.asarray(inputs["bn1_b"], np.float32)
    m1, v1 = np.asarray(inputs["bn1_m"], np.float32), np.asarray(inputs["bn1_v"], np.float32)
    fc1_w, fc1_b = np.asarray(inputs["fc1_w"], np.float32), np.asarray(inputs["fc1_b"], np.float32)
    g2, b2 = np.asarray(inputs["bn2_g"], np.float32), np.asarray(inputs["bn2_b"], np.float32)
    m2, v2 = np.asarray(inputs["bn2_m"], np.float32), np.asarray(inputs["bn2_v"], np.float32)
    fc2_w, fc2_b = np.asarray(inputs["fc2_w"], np.float32), np.asarray(inputs["fc2_b"], np.float32)
    g3, b3 = np.asarray(inputs["bn3_g"], np.float32), np.asarray(inputs["bn3_b"], np.float32)
    m3, v3 = np.asarray(inputs["bn3_m"], np.float32), np.asarray(inputs["bn3_v"], np.float32)
    final_w = np.asarray(inputs["final_w"], np.float32)
    final_b = np.asarray(inputs["final_b"], np.float32)

    a1 = g1 / np.sqrt(v1 + EPS)                      # [300] > 0
    c1 = a1 * (conv_b - m1) + b1                     # [300]
    a2 = g2 / np.sqrt(v2 + EPS)                      # [300,100]
    c2 = b2 - a2 * m2 + a2 * fc1_b                   # [300,100]
    a3 = g3 / np.sqrt(v3 + EPS)                      # [300]
    c3 = a3 * (fc2_b - m3) + b3                      # [300]

    bf = ml_dtypes.bfloat16

    # conv weights [77, 384]: rows 0..75 = a1*w (im2col row c*19+k),
    # row 76 = c1; cols >= 300 are zero pad
    wconv = np.zeros((CK, WCONV_COLS), np.float32)
    wconv[0:76, :N] = (conv_w * a1[:, None, None]).transpose(1, 2, 0).reshape(76, N)
    c1t = np.zeros((128, 3), np.float32)
    for ci in range(3):
        u0 = 128 * ci
        un = min(128, N - u0) if N - u0 < 128 else 128
        seg = c1[u0:min(u0 + 128, N)]
        c1t[0:len(seg), ci] = seg

    # fc1: lhsT [84, 128] per unit; rows 0..82 = a2*w1 (p-major),
    # row 83 = c2 (pairs with the ones row of pTall); h cols 100..127 zero
    w1aug = np.zeros((NPAD, LP + 1, HPAD), np.float32)
    w1aug[:N, :LP, :C1] = (fc1_w * a2[:, :, None]).transpose(0, 2, 1)
    w1aug[:N, LP, :C1] = c2

    # fc2: rhs [101, 1] per unit; rows 0..99 = a3*w2, row 100 = c3
    w2aug = np.zeros((NPAD, C1 + 1), np.float32)
    w2aug[:N, :C1] = fc2_w * a3[:, None]
    w2aug[:N, C1] = c3

    fwpad = np.zeros((NCLS, NPAD), np.float32)
    fwpad[:, :N] = final_w

    identity = np.eye(128, dtype=np.float32)

    in_maps = []
    for i in range(NCORES):
        us = slice(i * ULOC, (i + 1) * ULOC)
        w1c = w1aug[us].transpose(1, 0, 2).reshape(LP + 1, ULOC * HPAD)
        w2c = w2aug[us].T                                   # [101, 38]
        fwc = np.broadcast_to(fwpad[:, us], (128, NCLS, ULOC))
        in_maps.append({
            "xloc": np.ascontiguousarray(x[i * BLOC:(i + 1) * BLOC]).astype(bf),
            "wconv": wconv.astype(bf),
            "w1aug": np.ascontiguousarray(w1c).astype(bf),
            "w2aug": np.ascontiguousarray(w2c).astype(bf),
            "fwrep": np.ascontiguousarray(fwc).astype(bf),
            "ident": identity.astype(bf),
            "onesrow": np.ones((1, ULOC * B), np.float32).astype(bf),
            "c1p": c1t,
        })
    return in_maps, final_b


def kernel(**inputs):
    from concourse.bass_utils import run_bass_kernel_spmd

    if "nc" not in _CACHE:
        _CACHE["nc"] = _build_bass()
    nc = _CACHE["nc"]

    in_maps, final_b = _host_prep(inputs)
    res = run_bass_kernel_spmd(nc, in_maps, core_ids=list(range(NCORES)))
    out = np.zeros((B, NCLS), np.float32)
    for r in res.results:
        out += r["out_part"]
    out += final_b[None, :]
    return out


# revision 20
# speedup vs baseline: 1.3207x; 1.0001x over previous
"""ExplaiNN (dense_cnn) Trainium2 Bass kernel, 8-core SPMD. v2.

Pipeline per reference:
  conv1d(4->300 units, K=19) + BN1 + exp + maxpool(7) -> per-unit fc1 (83->100)
  + BN2 + relu -> per-unit fc2 (100->1) + BN3 + relu -> final linear (300->2).

Distribution: conv+pool batch-sharded (16 b/core, all units), then an AllToAll
exchanges pooled features so fc1/fc2/final run unit-sharded (38 u/core, full
batch 128).  Final [128,2] partials are summed on host.

v2 vs v1:
  - conv in bf16 (fp32r was ~4x slower + no FWL on LDWEIGHTS)
  - BN1 affine folded into conv weights (a1*w) + a ones-row carrying c1,
    so psum is already normalized; exp applied post-pool with no scale/bias
  - maxpool split across engines: DVE reduce_max direct from PSUM for some
    batches, DVE/ACT copy to SBUF + GpSimd pairwise-max tree for the rest
  - pexp -> poolT transpose via pipelined PE transposes (identity matmul)
    + batched DVE evacuation, replacing 48 serial DMA_TRANSPOSEs (60us of
    HWDGE sequencer occupancy)
  - fc1 weights padded 100->128 h cols so LDWEIGHTS gets FWL
"""

import numpy as np
import ml_dtypes

B, N, L, K, C1 = 128, 300, 600, 19, 100
PS = 7
LP = 83            # pool windows
LC2 = 582          # psum conv cols (581 needed, +1 garbage for even splits)
NCLS = 2
EPS = 1e-5

NCORES = 8
BLOC = B // NCORES            # 16 batch per core in phase A
NPAD = 304                    # units padded to 8*38
ULOC = NPAD // NCORES         # 38 units per core in phase B
CK = 76                       # 4*19 contraction rows
WCONV_COLS = 384              # conv weight cols padded so every matmul is M=128
QP = 96                       # pexp q-cols padded (83 pools + ones col at 83)
HPAD = 128                    # fc1 h padded 100->128 (FWL needs 128 weight cols)

# packed-weight column offsets
W_CONV, W_W1, W_W2, W_FW, W_ID = 0, 384, 5248, 5286, 5362
WTOT = 5490

# conv matmul column splits: even sizes, none crossing the 512-col (2KB)
# PSUM bank boundary of the 2-bank [128, 582] fp32 tile
CSPLIT = [(0, 294), (294, 218), (512, 70)]

# per-batch pool mode, same for each chunk: 'd'=DVE reduce_max direct,
# 'a'=ACT copy to SBUF (w-major) + DVE bf16 pairwise-max tree (2x mode)
POOL_MODES = "dddddddd" + "aaaaaaaa"

_CACHE = {}


def _build_bass():
    import concourse.bass as bass
    import concourse.bacc as bacc
    import concourse.mybir as mybir
    import concourse.tile as tile

    f32, bf16 = mybir.dt.float32, mybir.dt.bfloat16

    nc = bacc.Bacc("TRN2")
    xloc = nc.declare_dram_parameter("xloc", [BLOC, 4, L], bf16, isOutput=False)
    # packed bf16 weights: [wconv 0:384 | w1aug 384:5248 | w2aug 5248:5286 |
    #                       fwrep 5286:5362 | ident 5362:5490]
    wpack = nc.declare_dram_parameter("wpack", [128, WTOT], bf16, isOutput=False)
    c1p = nc.declare_dram_parameter("c1p", [128, 3], mybir.dt.float32, isOutput=False)
    out_part = nc.declare_dram_parameter("out_part", [B, NCLS], f32, isOutput=True)

    n_copy = sum(1 for m in POOL_MODES if m != 'd')   # copy-path slots per chunk
    b_copy0 = POOL_MODES.index('a')

    with tile.TileContext(nc) as tc:
        with (
            tc.tile_pool(name="dram", bufs=1, space="DRAM") as dram_pool,
            tc.tile_pool(name="singles", bufs=1) as singles,
            tc.tile_pool(name="im2col", bufs=1) as im2col_pool,
            tc.tile_pool(name="praw", bufs=1) as praw_pool,
            tc.tile_pool(name="praws", bufs=2) as praws_pool,
            tc.tile_pool(name="gpst", bufs=2) as gpst_pool,
            tc.tile_pool(name="pexp", bufs=1) as pexp_pool,
            tc.tile_pool(name="scratch", bufs=1, space="PSUM") as scratch_pool,
        ):
            # DRAM exchange buffers: [dest core, p-row, unit, batch]
            p2p_in = dram_pool.tile([NCORES, LP + 1, ULOC, BLOC], bf16,
                                    name="p2p_in")
            p2p_out = dram_pool.tile([NCORES, LP + 1, ULOC, BLOC], bf16,
                                     name="p2p_out")

            wp_sb = singles.tile([128, WTOT], bf16)
            nc.sync.dma_start(out=wp_sb[:, 0:W_W1], in_=wpack[:, 0:W_W1])
            nc.sync.dma_start(out=wp_sb[:, W_W1:WTOT], in_=wpack[:, W_W1:WTOT])
            c1_sb = singles.tile([128, 3], f32)
            nc.scalar.dma_start(out=c1_sb, in_=c1p[:, :])
            wconv_sb = wp_sb[0:CK, W_CONV:W_CONV + WCONV_COLS]
            ident_sb = wp_sb[0:128, W_ID:W_ID + 128]

            # im2col: one [76, 16, 582] bf16 tile; row (c*19+k), col (b, l)
            # <- x[b, c, l+k]  (l+k <= 599).  One 3-dim DMA per channel.
            im2all = im2col_pool.tile([CK, BLOC, LC2], bf16, name="im2all")
            for c in range(4):
                src = bass.AP(
                    tensor=xloc,
                    offset=c * L,
                    ap=[[1, K], [4 * L, BLOC], [1, LC2]],
                )
                nc.sync.dma_start(out=im2all[c * K:(c + 1) * K, :, :], in_=src)

            praw = []       # pooled, BN1-normalized, pre-exp [128, 16, 83]
            praw_s = []     # raw conv rows staged for the GpSimd tree
            pexp = []       # exp'd pooled features [128, 16, 96], col 83 = ones
            for ci in range(3):
                praw.append(praw_pool.tile([128, BLOC, LP], bf16, name=f"praw{ci}"))
                praw_s.append(praws_pool.tile([128, n_copy, PS * LP], bf16,
                                              name=f"praws", tag="praws"))
                p = pexp_pool.tile([128, BLOC, QP], bf16, name=f"pexp{ci}")
                nc.gpsimd.memset(p[:, :, LP:QP], 1.0)
                pexp.append(p)

            def absorb(tile_ap):
                s = scratch_pool.tile([2, 2], f32, name="dummy", tag="dummy")
                src = tile_ap.bitcast(bf16) if tile_ap.dtype != bf16 else tile_ap
                src = src[0:1, 0:2]
                nc.tensor.matmul(out=s, lhsT=src, rhs=src,
                                 start=True, stop=True)

            # ---------------- conv + pool dispatch ----------------
            with tc.tile_pool(name="psA", bufs=3, space="PSUM") as psum_pool:
                absorb(wconv_sb[0:1, 0:2])
                for ci in range(3):
                    u0 = 128 * ci
                    lhsT = wconv_sb[:, u0:u0 + 128]
                    slot = 0
                    for b in range(BLOC):
                        ps = psum_pool.tile([128, LC2], f32, name="ps", tag="ps")
                        for (l0, ncol) in CSPLIT:
                            nc.tensor.matmul(
                                out=ps[:, l0:l0 + ncol],
                                lhsT=lhsT,
                                rhs=im2all[:, b, l0:l0 + ncol],
                                start=True, stop=True,
                            )
                        mode = POOL_MODES[b]
                        if mode == 'd':
                            nc.vector.reduce_max(
                                out=praw[ci][:, b, :],
                                in_=ps[:, 0:581].rearrange("p (q w) -> p q w", w=PS),
                                axis=mybir.AxisListType.X,
                            )
                        else:
                            # copy in w-major order so the DVE tree below is
                            # contiguous (bf16 2x mode): col w*83+q <- psum 7q+w
                            nc.scalar.copy(
                                out=praw_s[ci][:, slot, :].rearrange(
                                    "p (w q) -> p w q", q=LP),
                                in_=ps[:, 0:581].rearrange("p (q w) -> p w q", w=PS),
                            )
                            slot += 1

                    # DVE bf16 pairwise-max tree over the copy-path batches
                    if n_copy:
                        s = praw_s[ci]
                        w_of = lambda w: s[:, :, w * LP:(w + 1) * LP]
                        tA = gpst_pool.tile([128, n_copy, LP], bf16, name="tA", tag="tA")
                        tB = gpst_pool.tile([128, n_copy, LP], bf16, name="tB", tag="tB")
                        tC = gpst_pool.tile([128, n_copy, LP], bf16, name="tC", tag="tC")
                        tD = gpst_pool.tile([128, n_copy, LP], bf16, name="tD", tag="tD")
                        tE = gpst_pool.tile([128, n_copy, LP], bf16, name="tE", tag="tE")
                        nc.vector.tensor_max(out=tA, in0=w_of(0), in1=w_of(1))
                        nc.vector.tensor_max(out=tB, in0=w_of(2), in1=w_of(3))
                        nc.vector.tensor_max(out=tC, in0=w_of(4), in1=w_of(5))
                        nc.vector.tensor_max(out=tD, in0=tA, in1=tB)
                        nc.vector.tensor_max(out=tE, in0=tC, in1=w_of(6))
                        nc.vector.tensor_max(
                            out=praw[ci][:, b_copy0:b_copy0 + n_copy, :],
                            in0=tD, in1=tE)

                    # exp over the chunk's pooled features (normalized already)
                    nc.scalar.activation(
                        out=pexp[ci][:, :, 0:LP],
                        in_=praw[ci][:, :, :],
                        func=mybir.ActivationFunctionType.Exp,
                        bias=c1_sb[:, ci:ci + 1],
                    )

            # ---------------- transpose via PE + evac ----------------
            # poolT[p, u, b] = pexp[u, b, p]; p-row 83 = ones (fc1 bias row)
            poolT = singles.tile([LP + 1, NPAD, BLOC], bf16)
            with tc.tile_pool(name="psT", bufs=3, space="PSUM") as psumt_pool:
                absorb(ident_sb[0:1, 0:2])
                for ci in range(3):
                    u0 = 128 * ci
                    un = min(128, NPAD - u0)       # 128,128,48
                    for b0 in range(0, BLOC, 4):
                        pst = psumt_pool.tile([QP, 4, 128], bf16, name="psT", tag="psT")
                        for k in range(4):
                            nc.tensor.transpose(
                                out=pst[:, k, :],
                                in_=pexp[ci][:, b0 + k, :],
                                identity=ident_sb[:, :],
                            )
                        evac_out = poolT[0:LP + 1, u0:u0 + un, b0:b0 + 4]
                        evac_in = pst[0:LP + 1, :, 0:un].rearrange("p b u -> p u b")
                        if (b0 // 4) % 2 == 0:
                            nc.vector.tensor_copy(out=evac_out, in_=evac_in)
                        else:
                            nc.scalar.copy(out=evac_out, in_=evac_in)

            # ---------------- exchange ----------------
            CH = (LP + 1) * ULOC * BLOC
            UB = ULOC * BLOC
            dst = bass.AP(
                tensor=p2p_in.tensor,
                offset=0,
                ap=[[UB, LP + 1], [CH, NCORES], [1, UB]],
            )
            stg_src = poolT[:, :, :].rearrange("p u b -> p (u b)").rearrange(
                "p (j s) -> p j s", j=NCORES)
            nc.sync.dma_start(out=dst, in_=stg_src)
            nc.gpsimd.collective_compute(
                "AllToAll",
                mybir.AluOpType.bypass,
                replica_groups=[list(range(NCORES))],
                ins=[p2p_in[:]],
                outs=[p2p_out[:]],
            )
            # received: [84, (core, uloc, b)] into pTall
            pTall = singles.tile([LP + 1, NCORES, ULOC, BLOC], bf16)
            src = bass.AP(
                tensor=p2p_out.tensor,
                offset=0,
                ap=[[UB, LP + 1], [CH, NCORES], [1, UB]],
            )
            nc.sync.dma_start(out=pTall[:, :, :, :], in_=src)

            # ---------------- fc1 ----------------
            w1_sb = wp_sb[0:LP + 1, W_W1:W_W1 + ULOC * HPAD]
            w2_sb = wp_sb[0:C1 + 1, W_W2:W_W2 + ULOC]
            fw_sb = wp_sb[0:128, W_FW:W_FW + NCLS * ULOC]

            h2_sb = singles.tile([128, ULOC * B], bf16)
            nc.gpsimd.memset(h2_sb[96:128, :], 1.0)   # row 100 = fc2 bias ones

            with tc.tile_pool(name="psB", bufs=3, space="PSUM") as psum_b:
                absorb(w1_sb[0:1, 0:2])
                ngroups = (ULOC + 3) // 4
                for g in range(ngroups):
                    un = min(4, ULOC - 4 * g)
                    psf = psum_b.tile([HPAD, 4, B], f32, name="psf", tag="psf")
                    for k in range(un):
                        u = 4 * g + k
                        nc.tensor.matmul(
                            out=psf[:, k, :],
                            lhsT=w1_sb[:, u * HPAD:(u + 1) * HPAD],
                            rhs=pTall[:, :, u, :],
                            start=True, stop=True,
                        )
                    nc.vector.tensor_scalar_max(
                        out=h2_sb[0:C1, 4 * g * B:(4 * g + un) * B],
                        in0=psf[0:C1, 0:un, :],
                        scalar1=0.0,
                    )

                # ---------------- fc2 ----------------
                absorb(w2_sb[0:1, 0:2])
                ps38 = psum_b.tile([B, ULOC], f32, name="ps38", tag="ps38",
                                   bufs=1)
                for u in range(ULOC):
                    nc.tensor.matmul(
                        out=ps38[:, u:u + 1],
                        lhsT=h2_sb[0:C1 + 1, u * B:(u + 1) * B],
                        rhs=w2_sb[:, u:u + 1],
                        start=True, stop=True,
                    )
                h3_sb = singles.tile([B, ULOC], bf16)
                nc.vector.tensor_scalar_max(out=h3_sb, in0=ps38, scalar1=0.0)

            # ---------------- final linear (partial over my units) ---------
            out_sb = singles.tile([B, NCLS], f32)
            prod = singles.tile([B, ULOC], f32)
            for cls in range(NCLS):
                nc.vector.tensor_mul(out=prod, in0=h3_sb, in1=fw_sb[:, cls * ULOC:(cls + 1) * ULOC])
                nc.vector.reduce_sum(
                    out=out_sb[:, cls:cls + 1], in_=prod,
                    axis=mybir.AxisListType.X,
                )
            nc.sync.dma_start(out=out_part[:, :], in_=out_sb)

    nc.finalize()
    return nc


def _host_prep(inputs):
    """Fold BN affines, pad units to 304, build per-core input maps."""
    x = np.asarray(inputs["x"], np.float32)
    conv_w = np.asarray(inputs["conv_w"], np.float32)
    conv_b = np.asarray(inputs["conv_b"], np.float32)
    g1, b1 = np.asarray(inputs["bn1_g"], np.float32), np.asarray(inputs["bn1_b"], np.float32)
    m1, v1 = np.asarray(inputs["bn1_m"], np.float32), np.asarray(inputs["bn1_v"], np.float32)
    fc1_w, fc1_b = np.asarray(inputs["fc1_w"], np.float32), np.asarray(inputs["fc1_b"], np.float32)
    g2, b2 = np.asarray(inputs["bn2_g"], np.float32), np.asarray(inputs["bn2_b"], np.float32)
    m2, v2 = np.asarray(inputs["bn2_m"], np.float32), np.asarray(inputs["bn2_v"], np.float32)
    fc2_w, fc2_b = np.asarray(inputs["fc2_w"], np.float32), np.asarray(inputs["fc2_b"], np.float32)
    g3, b3 = np.asarray(inputs["bn3_g"], np.float32), np.asarray(inputs["bn3_b"], np.float32)
    m3, v3 = np.asarray(inputs["bn3_m"], np.float32), np.asarray(inputs["bn3_v"], np.float32)
    final_w = np.asarray(inputs["final_w"], np.float32)
    final_b = np.asarray(inputs["final_b"], np.float32)

    a1 = g1 / np.sqrt(v1 + EPS)                      # [300] > 0
    c1 = a1 * (conv_b - m1) + b1                     # [300]
    a2 = g2 / np.sqrt(v2 + EPS)                      # [300,100]
    c2 = b2 - a2 * m2 + a2 * fc1_b                   # [300,100]
    a3 = g3 / np.sqrt(v3 + EPS)                      # [300]
    c3 = a3 * (fc2_b - m3) + b3                      # [300]

    bf = ml_dtypes.bfloat16

    # conv weights [76, 384]: a1 folded in; cols >= 300 are zero pad
    wconv = np.zeros((CK, WCONV_COLS), np.float32)
    wconv[0:76, :N] = (conv_w * a1[:, None, None]).transpose(1, 2, 0).reshape(76, N)
    c1t = np.zeros((128, 3), np.float32)
    for ci in range(3):
        u0 = 128 * ci
        seg = c1[u0:min(u0 + 128, N)]
        c1t[0:len(seg), ci] = seg

    # fc1: lhsT [84, 128] per unit; rows 0..82 = a2*w1 (p-major),
    # row 83 = c2 (pairs with the ones row of pTall); h cols 100..127 zero
    w1aug = np.zeros((NPAD, LP + 1, HPAD), np.float32)
    w1aug[:N, :LP, :C1] = (fc1_w * a2[:, :, None]).transpose(0, 2, 1)
    w1aug[:N, LP, :C1] = c2

    # fc2: rhs [101, 1] per unit; rows 0..99 = a3*w2, row 100 = c3
    w2aug = np.zeros((NPAD, C1 + 1), np.float32)
    w2aug[:N, :C1] = fc2_w * a3[:, None]
    w2aug[:N, C1] = c3

    fwpad = np.zeros((NCLS, NPAD), np.float32)
    fwpad[:, :N] = final_w

    identity = np.eye(128, dtype=np.float32)

    in_maps = []
    for i in range(NCORES):
        us = slice(i * ULOC, (i + 1) * ULOC)
        w1c = w1aug[us].transpose(1, 0, 2).reshape(LP + 1, ULOC * HPAD)
        w2c = w2aug[us].T                                   # [101, 38]
        wp = np.zeros((128, WTOT), np.float32)
        wp[0:CK, W_CONV:W_CONV + WCONV_COLS] = wconv
        wp[0:LP + 1, W_W1:W_W1 + ULOC * HPAD] = w1c
        wp[0:C1 + 1, W_W2:W_W2 + ULOC] = w2c
        for cls in range(NCLS):
            wp[:, W_FW + cls * ULOC:W_FW + (cls + 1) * ULOC] = fwpad[cls, us][None, :]
        wp[:, W_ID:W_ID + 128] = identity
        in_maps.append({
            "xloc": np.ascontiguousarray(x[i * BLOC:(i + 1) * BLOC]).astype(bf),
            "wpack": wp.astype(bf),
            "c1p": c1t,
        })
    return in_maps, final_b


def kernel(**inputs):
    from concourse.bass_utils import run_bass_kernel_spmd

    if "nc" not in _CACHE:
        _CACHE["nc"] = _build_bass()
    nc = _CACHE["nc"]

    in_maps, final_b = _host_prep(inputs)
    res = run_bass_kernel_spmd(nc, in_maps, core_ids=list(range(NCORES)))
    out = np.zeros((B, NCLS), np.float32)
    for r in res.results:
        out += r["out_part"]
    out += final_b[None, :]
    return out


# revision 22
# speedup vs baseline: 2.0333x; 1.5396x over previous
"""ExplaiNN (dense_cnn) Trainium2 Bass kernel, 8-core SPMD. v2.

Pipeline per reference:
  conv1d(4->300 units, K=19) + BN1 + exp + maxpool(7) -> per-unit fc1 (83->100)
  + BN2 + relu -> per-unit fc2 (100->1) + BN3 + relu -> final linear (300->2).

Distribution: conv+pool batch-sharded (16 b/core, all units), then an AllToAll
exchanges pooled features so fc1/fc2/final run unit-sharded (38 u/core, full
batch 128).  Final [128,2] partials are summed on host.

v2 vs v1:
  - conv in bf16 (fp32r was ~4x slower + no FWL on LDWEIGHTS)
  - BN1 affine folded into conv weights (a1*w) + a ones-row carrying c1,
    so psum is already normalized; exp applied post-pool with no scale/bias
  - maxpool split across engines: DVE reduce_max direct from PSUM for some
    batches, DVE/ACT copy to SBUF + GpSimd pairwise-max tree for the rest
  - pexp -> poolT transpose via pipelined PE transposes (identity matmul)
    + batched DVE evacuation, replacing 48 serial DMA_TRANSPOSEs (60us of
    HWDGE sequencer occupancy)
  - fc1 weights padded 100->128 h cols so LDWEIGHTS gets FWL
"""

import numpy as np
import ml_dtypes

B, N, L, K, C1 = 128, 300, 600, 19, 100
PS = 7
LP = 83            # pool windows
LC2 = 582          # psum conv cols (581 needed, +1 garbage for even splits)
NCLS = 2
EPS = 1e-5

NCORES = 8
BLOC = B // NCORES            # 16 batch per core in phase A
NPAD = 304                    # units padded to 8*38
ULOC = NPAD // NCORES         # 38 units per core in phase B
CK = 76                       # 4*19 contraction rows
WCONV_COLS = 384              # conv weight cols padded so every matmul is M=128
QP = 96                       # pexp q-cols padded (83 pools + ones col at 83)
HPAD = 128                    # fc1 h padded 100->128 (FWL needs 128 weight cols)

# packed-weight column offsets
W_CONV, W_W1, W_W2, W_FW, W_ID = 0, 384, 5248, 5286, 5362
WTOT = 5490

# conv matmul column splits: window-aligned (72 and 11 pool windows), each
# fits a single PSUM bank so two independent pools pipeline deeply
CSPLIT = [(0, 504), (504, 78)]

# per-batch pool mode, same for each chunk: 'd'=DVE reduce_max direct,
# 'a'=ACT copy to SBUF (w-major) + DVE bf16 pairwise-max tree (2x mode)
POOL_MODES = "dddddddd" + "aaaaaaaa"

_CACHE = {}


def _build_bass():
    import concourse.bass as bass
    import concourse.bacc as bacc
    import concourse.mybir as mybir
    import concourse.tile as tile

    f32, bf16 = mybir.dt.float32, mybir.dt.bfloat16

    nc = bacc.Bacc("TRN2")
    xloc = nc.declare_dram_parameter("xloc", [4, BLOC, L], bf16, isOutput=False)
    # packed bf16 weights: [wconv 0:384 | w1aug 384:5248 | w2aug 5248:5286 |
    #                       fwrep 5286:5362 | ident 5362:5490]
    wpack = nc.declare_dram_parameter("wpack", [128, WTOT], bf16, isOutput=False)
    c1p = nc.declare_dram_parameter("c1p", [128, 3], mybir.dt.float32, isOutput=False)
    out_part = nc.declare_dram_parameter("out_part", [B, NCLS], f32, isOutput=True)

    n_copy = sum(1 for m in POOL_MODES if m != 'd')   # copy-path slots per chunk
    b_copy0 = POOL_MODES.index('a')

    with tile.TileContext(nc) as tc:
        with (
            tc.tile_pool(name="dram", bufs=1, space="DRAM") as dram_pool,
            tc.tile_pool(name="singles", bufs=1) as singles,
            tc.tile_pool(name="im2col", bufs=1) as im2col_pool,
            tc.tile_pool(name="praw", bufs=1) as praw_pool,
            tc.tile_pool(name="praws", bufs=2) as praws_pool,
            tc.tile_pool(name="gpst", bufs=2) as gpst_pool,
            tc.tile_pool(name="pexp", bufs=1) as pexp_pool,
            tc.tile_pool(name="scratch", bufs=1, space="PSUM") as scratch_pool,
        ):
            # DRAM exchange buffers: [dest core, p-row, unit, batch]
            p2p_in = dram_pool.tile([NCORES, LP + 1, ULOC, BLOC], bf16,
                                    name="p2p_in")
            p2p_out = dram_pool.tile([NCORES, LP + 1, ULOC, BLOC], bf16,
                                     name="p2p_out")

            wp_sb = singles.tile([128, WTOT], bf16)
            nc.sync.dma_start(out=wp_sb[:, 0:W_W1], in_=wpack[:, 0:W_W1])
            nc.sync.dma_start(out=wp_sb[:, W_W1:WTOT], in_=wpack[:, W_W1:WTOT])
            c1_sb = singles.tile([128, 3], f32)
            nc.scalar.dma_start(out=c1_sb, in_=c1p[:, :])
            wconv_sb = wp_sb[0:CK, W_CONV:W_CONV + WCONV_COLS]
            ident_sb = wp_sb[0:128, W_ID:W_ID + 128]

            # im2col: [76, 16, 600] bf16; row (c*19+k), col (b, l) reads the
            # c-major flat x at 600b + l + k, one contiguous run per
            # partition (l+k <= 599 so no b-row crossing is ever read).
            im2all = im2col_pool.tile([CK, BLOC, L], bf16, name="im2all")
            NRUN = BLOC * L - (K - 1)
            src = bass.AP(
                tensor=xloc,
                offset=0,
                ap=[[BLOC * L, 4], [1, K], [1, NRUN]],
            )
            nc.sync.dma_start(
                out=im2all[:, :, :].rearrange("p b l -> p (b l)")[:, 0:NRUN],
                in_=src)

            praw = []       # pooled, BN1-normalized, pre-exp [128, 16, 83]
            praw_s = []     # raw conv rows staged for the GpSimd tree
            pexp = []       # exp'd pooled features [128, 16, 96], col 83 = ones
            for ci in range(3):
                praw.append(praw_pool.tile([128, BLOC, LP], bf16, name=f"praw{ci}"))
                praw_s.append(praws_pool.tile([128, n_copy, PS * LP], bf16,
                                              name=f"praws", tag="praws"))
                p = pexp_pool.tile([128, BLOC, QP], bf16, name=f"pexp{ci}")
                nc.gpsimd.memset(p[:, :, LP:QP], 1.0)
                pexp.append(p)

            def absorb(tile_ap):
                s = scratch_pool.tile([2, 2], f32, name="dummy", tag="dummy")
                src = tile_ap.bitcast(bf16) if tile_ap.dtype != bf16 else tile_ap
                src = src[0:1, 0:2]
                nc.tensor.matmul(out=s, lhsT=src, rhs=src,
                                 start=True, stop=True)

            # ---------------- conv + pool dispatch ----------------
            with (
                tc.tile_pool(name="psA0", bufs=3, space="PSUM") as pool_a,
                tc.tile_pool(name="psA1", bufs=3, space="PSUM") as pool_b,
            ):
                absorb(wconv_sb[0:1, 0:2])
                for ci in range(3):
                    u0 = 128 * ci
                    lhsT = wconv_sb[:, u0:u0 + 128]
                    slot = 0
                    for b in range(BLOC):
                        ps0 = pool_a.tile([128, 504], f32, name="ps0", tag="ps0")
                        ps1 = pool_b.tile([128, 78], f32, name="ps1", tag="ps1")
                        for pst, (l0, ncol) in zip((ps0, ps1), CSPLIT):
                            nc.tensor.matmul(
                                out=pst[:, :],
                                lhsT=lhsT,
                                rhs=im2all[:, b, l0:l0 + ncol],
                                start=True, stop=True,
                            )
                        mode = POOL_MODES[b]
                        if mode == 'd':
                            nc.vector.reduce_max(
                                out=praw[ci][:, b, 0:72],
                                in_=ps0[:, :].rearrange("p (q w) -> p q w", w=PS),
                                axis=mybir.AxisListType.X,
                            )
                            nc.vector.reduce_max(
                                out=praw[ci][:, b, 72:LP],
                                in_=ps1[:, 0:77].rearrange("p (q w) -> p q w", w=PS),
                                axis=mybir.AxisListType.X,
                            )
                        else:
                            # copy in w-major order so the DVE tree below is
                            # contiguous (bf16 2x mode): col w*83+q <- psum 7q+w
                            nc.scalar.copy(
                                out=praw_s[ci][:, slot, :].rearrange(
                                    "p (w q) -> p w q", q=LP)[:, :, 0:72],
                                in_=ps0[:, :].rearrange("p (q w) -> p w q", w=PS),
                            )
                            nc.scalar.copy(
                                out=praw_s[ci][:, slot, :].rearrange(
                                    "p (w q) -> p w q", q=LP)[:, :, 72:LP],
                                in_=ps1[:, 0:77].rearrange("p (q w) -> p w q", w=PS),
                            )
                            slot += 1

                    # DVE bf16 pairwise-max tree over the copy-path batches
                    if n_copy:
                        s = praw_s[ci]
                        w_of = lambda w: s[:, :, w * LP:(w + 1) * LP]
                        tA = gpst_pool.tile([128, n_copy, LP], bf16, name="tA", tag="tA")
                        tB = gpst_pool.tile([128, n_copy, LP], bf16, name="tB", tag="tB")
                        tC = gpst_pool.tile([128, n_copy, LP], bf16, name="tC", tag="tC")
                        tD = gpst_pool.tile([128, n_copy, LP], bf16, name="tD", tag="tD")
                        tE = gpst_pool.tile([128, n_copy, LP], bf16, name="tE", tag="tE")
                        nc.vector.tensor_max(out=tA, in0=w_of(0), in1=w_of(1))
                        nc.vector.tensor_max(out=tB, in0=w_of(2), in1=w_of(3))
                        nc.vector.tensor_max(out=tC, in0=w_of(4), in1=w_of(5))
                        nc.vector.tensor_max(out=tD, in0=tA, in1=tB)
                        nc.vector.tensor_max(out=tE, in0=tC, in1=w_of(6))
                        nc.vector.tensor_max(
                            out=praw[ci][:, b_copy0:b_copy0 + n_copy, :],
                            in0=tD, in1=tE)

                    # exp over the chunk's pooled features (normalized already)
                    nc.scalar.activation(
                        out=pexp[ci][:, :, 0:LP],
                        in_=praw[ci][:, :, :],
                        func=mybir.ActivationFunctionType.Exp,
                        bias=c1_sb[:, ci:ci + 1],
                    )

            # ---------------- transpose via PE + evac ----------------
            # poolT[p, u, b] = pexp[u, b, p]; p-row 83 = ones (fc1 bias row)
            poolT = singles.tile([LP + 1, NPAD, BLOC], bf16)
            with tc.tile_pool(name="psT", bufs=3, space="PSUM") as psumt_pool:
                absorb(ident_sb[0:1, 0:2])
                for ci in range(3):
                    u0 = 128 * ci
                    un = min(128, NPAD - u0)       # 128,128,48
                    for b0 in range(0, BLOC, 4):
                        pst = psumt_pool.tile([QP, 4, 128], bf16, name="psT", tag="psT")
                        for k in range(4):
                            nc.tensor.transpose(
                                out=pst[:, k, :],
                                in_=pexp[ci][:, b0 + k, :],
                                identity=ident_sb[:, :],
                            )
                        evac_out = poolT[0:LP + 1, u0:u0 + un, b0:b0 + 4]
                        evac_in = pst[0:LP + 1, :, 0:un].rearrange("p b u -> p u b")
                        if (b0 // 4) % 2 == 0:
                            nc.vector.tensor_copy(out=evac_out, in_=evac_in)
                        else:
                            nc.scalar.copy(out=evac_out, in_=evac_in)

            # ---------------- exchange ----------------
            CH = (LP + 1) * ULOC * BLOC
            UB = ULOC * BLOC
            dst = bass.AP(
                tensor=p2p_in.tensor,
                offset=0,
                ap=[[UB, LP + 1], [CH, NCORES], [1, UB]],
            )
            stg_src = poolT[:, :, :].rearrange("p u b -> p (u b)").rearrange(
                "p (j s) -> p j s", j=NCORES)
            nc.sync.dma_start(out=dst, in_=stg_src)
            nc.gpsimd.collective_compute(
                "AllToAll",
                mybir.AluOpType.bypass,
                replica_groups=[list(range(NCORES))],
                ins=[p2p_in[:]],
                outs=[p2p_out[:]],
            )
            # received: [84, (core, uloc, b)] into pTall
            pTall = singles.tile([LP + 1, NCORES, ULOC, BLOC], bf16)
            src = bass.AP(
                tensor=p2p_out.tensor,
                offset=0,
                ap=[[UB, LP + 1], [CH, NCORES], [1, UB]],
            )
            nc.sync.dma_start(out=pTall[:, :, :, :], in_=src)

            # ---------------- fc1 ----------------
            w1_sb = wp_sb[0:LP + 1, W_W1:W_W1 + ULOC * HPAD]
            w2_sb = wp_sb[0:C1 + 1, W_W2:W_W2 + ULOC]
            fw_sb = wp_sb[0:128, W_FW:W_FW + NCLS * ULOC]

            h2_sb = singles.tile([128, ULOC * B], bf16)
            nc.gpsimd.memset(h2_sb[96:128, :], 1.0)   # row 100 = fc2 bias ones

            with tc.tile_pool(name="psB", bufs=3, space="PSUM") as psum_b:
                absorb(w1_sb[0:1, 0:2])
                ngroups = (ULOC + 3) // 4
                for g in range(ngroups):
                    un = min(4, ULOC - 4 * g)
                    psf = psum_b.tile([HPAD, 4, B], f32, name="psf", tag="psf")
                    for k in range(un):
                        u = 4 * g + k
                        nc.tensor.matmul(
                            out=psf[:, k, :],
                            lhsT=w1_sb[:, u * HPAD:(u + 1) * HPAD],
                            rhs=pTall[:, :, u, :],
                            start=True, stop=True,
                        )
                    nc.vector.tensor_scalar_max(
                        out=h2_sb[0:C1, 4 * g * B:(4 * g + un) * B],
                        in0=psf[0:C1, 0:un, :],
                        scalar1=0.0,
                    )

                # ---------------- fc2 ----------------
                absorb(w2_sb[0:1, 0:2])
                ps38 = psum_b.tile([B, ULOC], f32, name="ps38", tag="ps38",
                                   bufs=1)
                for u in range(ULOC):
                    nc.tensor.matmul(
                        out=ps38[:, u:u + 1],
                        lhsT=h2_sb[0:C1 + 1, u * B:(u + 1) * B],
                        rhs=w2_sb[:, u:u + 1],
                        start=True, stop=True,
                    )
                h3_sb = singles.tile([B, ULOC], bf16)
                nc.vector.tensor_scalar_max(out=h3_sb, in0=ps38, scalar1=0.0)

            # ---------------- final linear (partial over my units) ---------
            out_sb = singles.tile([B, NCLS], f32)
            prod = singles.tile([B, ULOC], f32)
            for cls in range(NCLS):
                nc.vector.tensor_mul(out=prod, in0=h3_sb, in1=fw_sb[:, cls * ULOC:(cls + 1) * ULOC])
                nc.vector.reduce_sum(
                    out=out_sb[:, cls:cls + 1], in_=prod,
                    axis=mybir.AxisListType.X,
                )
            nc.sync.dma_start(out=out_part[:, :], in_=out_sb)

    nc.finalize()
    return nc


def _host_prep(inputs):
    """Fold BN affines, pad units to 304, build per-core input maps."""
    x = np.asarray(inputs["x"], np.float32)
    conv_w = np.asarray(inputs["conv_w"], np.float32)
    conv_b = np.asarray(inputs["conv_b"], np.float32)
    g1, b1 = np.asarray(inputs["bn1_g"], np.float32), np.asarray(inputs["bn1_b"], np.float32)
    m1, v1 = np.asarray(inputs["bn1_m"], np.float32), np.asarray(inputs["bn1_v"], np.float32)
    fc1_w, fc1_b = np.asarray(inputs["fc1_w"], np.float32), np.asarray(inputs["fc1_b"], np.float32)
    g2, b2 = np.asarray(inputs["bn2_g"], np.float32), np.asarray(inputs["bn2_b"], np.float32)
    m2, v2 = np.asarray(inputs["bn2_m"], np.float32), np.asarray(inputs["bn2_v"], np.float32)
    fc2_w, fc2_b = np.asarray(inputs["fc2_w"], np.float32), np.asarray(inputs["fc2_b"], np.float32)
    g3, b3 = np.asarray(inputs["bn3_g"], np.float32), np.asarray(inputs["bn3_b"], np.float32)
    m3, v3 = np.asarray(inputs["bn3_m"], np.float32), np.asarray(inputs["bn3_v"], np.float32)
    final_w = np.asarray(inputs["final_w"], np.float32)
    final_b = np.asarray(inputs["final_b"], np.float32)

    a1 = g1 / np.sqrt(v1 + EPS)                      # [300] > 0
    c1 = a1 * (conv_b - m1) + b1                     # [300]
    a2 = g2 / np.sqrt(v2 + EPS)                      # [300,100]
    c2 = b2 - a2 * m2 + a2 * fc1_b                   # [300,100]
    a3 = g3 / np.sqrt(v3 + EPS)                      # [300]
    c3 = a3 * (fc2_b - m3) + b3                      # [300]

    bf = ml_dtypes.bfloat16

    # conv weights [76, 384]: a1 folded in; cols >= 300 are zero pad
    wconv = np.zeros((CK, WCONV_COLS), np.float32)
    wconv[0:76, :N] = (conv_w * a1[:, None, None]).transpose(1, 2, 0).reshape(76, N)
    c1t = np.zeros((128, 3), np.float32)
    for ci in range(3):
        u0 = 128 * ci
        seg = c1[u0:min(u0 + 128, N)]
        c1t[0:len(seg), ci] = seg

    # fc1: lhsT [84, 128] per unit; rows 0..82 = a2*w1 (p-major),
    # row 83 = c2 (pairs with the ones row of pTall); h cols 100..127 zero
    w1aug = np.zeros((NPAD, LP + 1, HPAD), np.float32)
    w1aug[:N, :LP, :C1] = (fc1_w * a2[:, :, None]).transpose(0, 2, 1)
    w1aug[:N, LP, :C1] = c2

    # fc2: rhs [101, 1] per unit; rows 0..99 = a3*w2, row 100 = c3
    w2aug = np.zeros((NPAD, C1 + 1), np.float32)
    w2aug[:N, :C1] = fc2_w * a3[:, None]
    w2aug[:N, C1] = c3

    fwpad = np.zeros((NCLS, NPAD), np.float32)
    fwpad[:, :N] = final_w

    identity = np.eye(128, dtype=np.float32)

    in_maps = []
    for i in range(NCORES):
        us = slice(i * ULOC, (i + 1) * ULOC)
        w1c = w1aug[us].transpose(1, 0, 2).reshape(LP + 1, ULOC * HPAD)
        w2c = w2aug[us].T                                   # [101, 38]
        wp = np.zeros((128, WTOT), np.float32)
        wp[0:CK, W_CONV:W_CONV + WCONV_COLS] = wconv
        wp[0:LP + 1, W_W1:W_W1 + ULOC * HPAD] = w1c
        wp[0:C1 + 1, W_W2:W_W2 + ULOC] = w2c
        for cls in range(NCLS):
            wp[:, W_FW + cls * ULOC:W_FW + (cls + 1) * ULOC] = fwpad[cls, us][None, :]
        wp[:, W_ID:W_ID + 128] = identity
        in_maps.append({
            "xloc": np.ascontiguousarray(x[i * BLOC:(i + 1) * BLOC].transpose(1, 0, 2)).astype(bf),
            "wpack": wp.astype(bf),
            "c1p": c1t,
        })
    return in_maps, final_b


def kernel(**inputs):
    from concourse.bass_utils import run_bass_kernel_spmd

    if "nc" not in _CACHE:
        _CACHE["nc"] = _build_bass()
    nc = _CACHE["nc"]

    in_maps, final_b = _host_prep(inputs)
    res = run_bass_kernel_spmd(nc, in_maps, core_ids=list(range(NCORES)))
    out = np.zeros((B, NCLS), np.float32)
    for r in res.results:
        out += r["out_part"]
    out += final_b[None, :]
    return out


# revision 23
# speedup vs baseline: 2.1464x; 1.0556x over previous
"""ExplaiNN (dense_cnn) Trainium2 Bass kernel, 8-core SPMD. v2.

Pipeline per reference:
  conv1d(4->300 units, K=19) + BN1 + exp + maxpool(7) -> per-unit fc1 (83->100)
  + BN2 + relu -> per-unit fc2 (100->1) + BN3 + relu -> final linear (300->2).

Distribution: conv+pool batch-sharded (16 b/core, all units), then an AllToAll
exchanges pooled features so fc1/fc2/final run unit-sharded (38 u/core, full
batch 128).  Final [128,2] partials are summed on host.

v2 vs v1:
  - conv in bf16 (fp32r was ~4x slower + no FWL on LDWEIGHTS)
  - BN1 affine folded into conv weights (a1*w) + a ones-row carrying c1,
    so psum is already normalized; exp applied post-pool with no scale/bias
  - maxpool split across engines: DVE reduce_max direct from PSUM for some
    batches, DVE/ACT copy to SBUF + GpSimd pairwise-max tree for the rest
  - pexp -> poolT transpose via pipelined PE transposes (identity matmul)
    + batched DVE evacuation, replacing 48 serial DMA_TRANSPOSEs (60us of
    HWDGE sequencer occupancy)
  - fc1 weights padded 100->128 h cols so LDWEIGHTS gets FWL
"""

import numpy as np
import ml_dtypes

B, N, L, K, C1 = 128, 300, 600, 19, 100
PS = 7
LP = 83            # pool windows
LC2 = 582          # psum conv cols (581 needed, +1 garbage for even splits)
NCLS = 2
EPS = 1e-5

NCORES = 8
BLOC = B // NCORES            # 16 batch per core in phase A
NPAD = 304                    # units padded to 8*38
ULOC = NPAD // NCORES         # 38 units per core in phase B
CK = 76                       # 4*19 contraction rows
WCONV_COLS = 384              # conv weight cols padded so every matmul is M=128
QP = 96                       # pexp q-cols padded (83 pools + ones col at 83)
HPAD = 128                    # fc1 h padded 100->128 (FWL needs 128 weight cols)

# packed-weight column offsets
W_CONV, W_W1, W_W2, W_FW, W_ID = 0, 384, 5248, 5286, 5362
WTOT = 5490

# conv matmul column splits for batch-pairs: window-aligned 36/36/11 pool
# windows, each [128, 2, n] fp32 tile fits a single PSUM bank
CSPLIT = [(0, 252), (252, 252), (504, 78)]

# per-batch-PAIR pool mode (uniform within a pair): 'd'=DVE reduce_max
# direct, 'a'=ACT copy to SBUF (w-major) + DVE bf16 pairwise-max tree
POOL_MODES = "dddddd" + "aaaaaaaaaa"

_CACHE = {}


def _build_bass():
    import concourse.bass as bass
    import concourse.bacc as bacc
    import concourse.mybir as mybir
    import concourse.tile as tile

    f32, bf16 = mybir.dt.float32, mybir.dt.bfloat16

    nc = bacc.Bacc("TRN2")
    xloc = nc.declare_dram_parameter("xloc", [4, BLOC, L], bf16, isOutput=False)
    # packed bf16 weights: [wconv 0:384 | w1aug 384:5248 | w2aug 5248:5286 |
    #                       fwrep 5286:5362 | ident 5362:5490]
    wpack = nc.declare_dram_parameter("wpack", [128, WTOT], bf16, isOutput=False)
    c1p = nc.declare_dram_parameter("c1p", [128, 3], mybir.dt.float32, isOutput=False)
    out_part = nc.declare_dram_parameter("out_part", [B, NCLS], f32, isOutput=True)

    n_copy = sum(1 for m in POOL_MODES if m != 'd')   # copy-path slots per chunk
    b_copy0 = POOL_MODES.index('a')

    with tile.TileContext(nc) as tc:
        with (
            tc.tile_pool(name="dram", bufs=1, space="DRAM") as dram_pool,
            tc.tile_pool(name="singles", bufs=1) as singles,
            tc.tile_pool(name="im2col", bufs=1) as im2col_pool,
            tc.tile_pool(name="praw", bufs=1) as praw_pool,
            tc.tile_pool(name="praws", bufs=2) as praws_pool,
            tc.tile_pool(name="gpst", bufs=2) as gpst_pool,
            tc.tile_pool(name="pexp", bufs=1) as pexp_pool,
            tc.tile_pool(name="scratch", bufs=1, space="PSUM") as scratch_pool,
        ):
            # DRAM exchange buffers: [dest core, p-row, unit, batch]
            p2p_in = dram_pool.tile([NCORES, LP + 1, ULOC, BLOC], bf16,
                                    name="p2p_in")
            p2p_out = dram_pool.tile([NCORES, LP + 1, ULOC, BLOC], bf16,
                                     name="p2p_out")

            # im2col: [76, 16, 600] bf16; row (c*19+k), col (b, l) reads the
            # c-major flat x at 600b + l + k, one contiguous run per
            # partition (l+k <= 599 so no b-row crossing is ever read).
            # Issued first: conv blocks on it.
            im2all = im2col_pool.tile([CK, BLOC, L], bf16, name="im2all")
            NRUN = BLOC * L - (K - 1)
            src = bass.AP(
                tensor=xloc,
                offset=0,
                ap=[[BLOC * L, 4], [1, K], [1, NRUN]],
            )
            nc.sync.dma_start(
                out=im2all[:, :, :].rearrange("p b l -> p (b l)")[:, 0:NRUN],
                in_=src)

            wp_sb = singles.tile([128, WTOT], bf16)
            nc.sync.dma_start(out=wp_sb[:, 0:W_W1], in_=wpack[:, 0:W_W1])
            c1_sb = singles.tile([128, 3], f32)
            nc.scalar.dma_start(out=c1_sb, in_=c1p[:, :])
            nc.sync.dma_start(out=wp_sb[:, W_W1:WTOT], in_=wpack[:, W_W1:WTOT])
            wconv_sb = wp_sb[0:CK, W_CONV:W_CONV + WCONV_COLS]
            ident_sb = wp_sb[0:128, W_ID:W_ID + 128]

            praw = []       # pooled, BN1-normalized, pre-exp [128, 16, 83]
            praw_s = []     # raw conv rows staged for the GpSimd tree
            pexp = []       # exp'd pooled features [128, 16, 96], col 83 = ones
            for ci in range(3):
                praw.append(praw_pool.tile([128, BLOC, LP], bf16, name=f"praw{ci}"))
                praw_s.append(praws_pool.tile([128, n_copy, PS * LP], bf16,
                                              name=f"praws", tag="praws"))
                p = pexp_pool.tile([128, BLOC, QP], bf16, name=f"pexp{ci}")
                nc.vector.memset(p[:, :, LP:QP], 1.0)
                pexp.append(p)

            def absorb(tile_ap):
                s = scratch_pool.tile([2, 2], f32, name="dummy", tag="dummy")
                src = tile_ap.bitcast(bf16) if tile_ap.dtype != bf16 else tile_ap
                src = src[0:1, 0:2]
                nc.tensor.matmul(out=s, lhsT=src, rhs=src,
                                 start=True, stop=True)

            # ---------------- conv + pool dispatch ----------------
            with (
                tc.tile_pool(name="psA0", bufs=2, space="PSUM") as pool_a,
                tc.tile_pool(name="psA1", bufs=2, space="PSUM") as pool_b,
                tc.tile_pool(name="psA2", bufs=2, space="PSUM") as pool_c,
            ):
                absorb(wconv_sb[0:1, 0:2])
                QSPLIT = [(0, 36), (36, 36), (72, 11)]   # window ranges
                for ci in range(3):
                    u0 = 128 * ci
                    lhsT = wconv_sb[:, u0:u0 + 128]
                    slot = 0
                    for bp in range(0, BLOC, 2):
                        pss = [
                            pool_a.tile([128, 2, 252], f32, name="ps0", tag="ps0"),
                            pool_b.tile([128, 2, 252], f32, name="ps1", tag="ps1"),
                            pool_c.tile([128, 2, 78], f32, name="ps2", tag="ps2"),
                        ]
                        for pst, (l0, ncol) in zip(pss, CSPLIT):
                            nc.tensor.matmul(
                                out=pst[:, :, :],
                                lhsT=lhsT,
                                rhs=im2all[:, bp:bp + 2, l0:l0 + ncol],
                                start=True, stop=True,
                            )
                        mode = POOL_MODES[bp]
                        if mode == 'd':
                            for pst, (q0, nq) in zip(pss, QSPLIT):
                                nc.vector.reduce_max(
                                    out=praw[ci][:, bp:bp + 2, q0:q0 + nq],
                                    in_=pst[:, :, 0:nq * PS].rearrange(
                                        "p s (q w) -> p s q w", w=PS),
                                    axis=mybir.AxisListType.X,
                                )
                        else:
                            # copy in w-major order so the DVE tree below is
                            # contiguous (bf16 2x): col w*83+q <- psum 7q+w
                            view = praw_s[ci][:, slot:slot + 2, :].rearrange(
                                "p s (w q) -> p s w q", q=LP)
                            for pst, (q0, nq) in zip(pss, QSPLIT):
                                nc.scalar.copy(
                                    out=view[:, :, :, q0:q0 + nq],
                                    in_=pst[:, :, 0:nq * PS].rearrange(
                                        "p s (q w) -> p s w q", w=PS),
                                )
                            slot += 2

                    # DVE bf16 pairwise-max tree over the copy-path batches
                    if n_copy:
                        s = praw_s[ci]
                        w_of = lambda w: s[:, :, w * LP:(w + 1) * LP]
                        tA = gpst_pool.tile([128, n_copy, LP], bf16, name="tA", tag="tA")
                        tB = gpst_pool.tile([128, n_copy, LP], bf16, name="tB", tag="tB")
                        tC = gpst_pool.tile([128, n_copy, LP], bf16, name="tC", tag="tC")
                        tD = gpst_pool.tile([128, n_copy, LP], bf16, name="tD", tag="tD")
                        tE = gpst_pool.tile([128, n_copy, LP], bf16, name="tE", tag="tE")
                        nc.vector.tensor_max(out=tA, in0=w_of(0), in1=w_of(1))
                        nc.vector.tensor_max(out=tB, in0=w_of(2), in1=w_of(3))
                        nc.vector.tensor_max(out=tC, in0=w_of(4), in1=w_of(5))
                        nc.vector.tensor_max(out=tD, in0=tA, in1=tB)
                        nc.vector.tensor_max(out=tE, in0=tC, in1=w_of(6))
                        nc.vector.tensor_max(
                            out=praw[ci][:, b_copy0:b_copy0 + n_copy, :],
                            in0=tD, in1=tE)

                    # exp over the chunk's pooled features (normalized already)
                    nc.scalar.activation(
                        out=pexp[ci][:, :, 0:LP],
                        in_=praw[ci][:, :, :],
                        func=mybir.ActivationFunctionType.Exp,
                        bias=c1_sb[:, ci:ci + 1],
                    )

            # ---------------- transpose via PE + evac ----------------
            # poolT[p, u, b] = pexp[u, b, p]; p-row 83 = ones (fc1 bias row)
            poolT = singles.tile([LP + 1, NPAD, BLOC], bf16)
            with tc.tile_pool(name="psT", bufs=3, space="PSUM") as psumt_pool:
                absorb(ident_sb[0:1, 0:2])
                for ci in range(3):
                    u0 = 128 * ci
                    un = min(128, NPAD - u0)       # 128,128,48
                    for b0 in range(0, BLOC, 4):
                        pst = psumt_pool.tile([QP, 4, 128], bf16, name="psT", tag="psT")
                        for k in range(4):
                            nc.tensor.transpose(
                                out=pst[:, k, :],
                                in_=pexp[ci][:, b0 + k, :],
                                identity=ident_sb[:, :],
                            )
                        evac_out = poolT[0:LP + 1, u0:u0 + un, b0:b0 + 4]
                        evac_in = pst[0:LP + 1, :, 0:un].rearrange("p b u -> p u b")
                        if (b0 // 4) % 2 == 0:
                            nc.vector.tensor_copy(out=evac_out, in_=evac_in)
                        else:
                            nc.scalar.copy(out=evac_out, in_=evac_in)

            # ---------------- exchange ----------------
            CH = (LP + 1) * ULOC * BLOC
            UB = ULOC * BLOC
            dst = bass.AP(
                tensor=p2p_in.tensor,
                offset=0,
                ap=[[UB, LP + 1], [CH, NCORES], [1, UB]],
            )
            stg_src = poolT[:, :, :].rearrange("p u b -> p (u b)").rearrange(
                "p (j s) -> p j s", j=NCORES)
            nc.sync.dma_start(out=dst, in_=stg_src)
            nc.gpsimd.collective_compute(
                "AllToAll",
                mybir.AluOpType.bypass,
                replica_groups=[list(range(NCORES))],
                ins=[p2p_in[:]],
                outs=[p2p_out[:]],
            )
            # received: [84, (core, uloc, b)] into pTall
            pTall = singles.tile([LP + 1, NCORES, ULOC, BLOC], bf16)
            src = bass.AP(
                tensor=p2p_out.tensor,
                offset=0,
                ap=[[UB, LP + 1], [CH, NCORES], [1, UB]],
            )
            nc.sync.dma_start(out=pTall[:, :, :, :], in_=src)

            # ---------------- fc1 ----------------
            w1_sb = wp_sb[0:LP + 1, W_W1:W_W1 + ULOC * HPAD]
            w2_sb = wp_sb[0:C1 + 1, W_W2:W_W2 + ULOC]
            fw_sb = wp_sb[0:128, W_FW:W_FW + NCLS * ULOC]

            h2_sb = singles.tile([128, ULOC * B], bf16)
            nc.vector.memset(h2_sb[96:128, :], 1.0)   # row 100 = fc2 bias ones

            with tc.tile_pool(name="psB", bufs=3, space="PSUM") as psum_b:
                absorb(w1_sb[0:1, 0:2])
                ngroups = (ULOC + 3) // 4
                for g in range(ngroups):
                    un = min(4, ULOC - 4 * g)
                    psf = psum_b.tile([HPAD, 4, B], f32, name="psf", tag="psf")
                    for k in range(un):
                        u = 4 * g + k
                        nc.tensor.matmul(
                            out=psf[:, k, :],
                            lhsT=w1_sb[:, u * HPAD:(u + 1) * HPAD],
                            rhs=pTall[:, :, u, :],
                            start=True, stop=True,
                        )
                    if g % 2 == 0:
                        nc.vector.tensor_scalar_max(
                            out=h2_sb[0:C1, 4 * g * B:(4 * g + un) * B],
                            in0=psf[0:C1, 0:un, :],
                            scalar1=0.0,
                        )
                    else:
                        nc.scalar.activation(
                            out=h2_sb[0:C1, 4 * g * B:(4 * g + un) * B],
                            in_=psf[0:C1, 0:un, :],
                            func=mybir.ActivationFunctionType.Relu,
                        )

                # ---------------- fc2 ----------------
                absorb(w2_sb[0:1, 0:2])
                ps38 = psum_b.tile([B, ULOC], f32, name="ps38", tag="ps38",
                                   bufs=1)
                for u in range(ULOC):
                    nc.tensor.matmul(
                        out=ps38[:, u:u + 1],
                        lhsT=h2_sb[0:C1 + 1, u * B:(u + 1) * B],
                        rhs=w2_sb[:, u:u + 1],
                        start=True, stop=True,
                    )
                h3_sb = singles.tile([B, ULOC], bf16)
                nc.vector.tensor_scalar_max(out=h3_sb, in0=ps38, scalar1=0.0)

            # ---------------- final linear (partial over my units) ---------
            out_sb = singles.tile([B, NCLS], f32)
            prod = singles.tile([B, ULOC], f32)
            for cls in range(NCLS):
                nc.vector.tensor_mul(out=prod, in0=h3_sb, in1=fw_sb[:, cls * ULOC:(cls + 1) * ULOC])
                nc.vector.reduce_sum(
                    out=out_sb[:, cls:cls + 1], in_=prod,
                    axis=mybir.AxisListType.X,
                )
            nc.sync.dma_start(out=out_part[:, :], in_=out_sb)

    nc.finalize()
    return nc


def _host_prep(inputs):
    """Fold BN affines, pad units to 304, build per-core input maps."""
    x = np.asarray(inputs["x"], np.float32)
    conv_w = np.asarray(inputs["conv_w"], np.float32)
    conv_b = np.asarray(inputs["conv_b"], np.float32)
    g1, b1 = np.asarray(inputs["bn1_g"], np.float32), np.asarray(inputs["bn1_b"], np.float32)
    m1, v1 = np.asarray(inputs["bn1_m"], np.float32), np.asarray(inputs["bn1_v"], np.float32)
    fc1_w, fc1_b = np.asarray(inputs["fc1_w"], np.float32), np.asarray(inputs["fc1_b"], np.float32)
    g2, b2 = np.asarray(inputs["bn2_g"], np.float32), np.asarray(inputs["bn2_b"], np.float32)
    m2, v2 = np.asarray(inputs["bn2_m"], np.float32), np.asarray(inputs["bn2_v"], np.float32)
    fc2_w, fc2_b = np.asarray(inputs["fc2_w"], np.float32), np.asarray(inputs["fc2_b"], np.float32)
    g3, b3 = np.asarray(inputs["bn3_g"], np.float32), np.asarray(inputs["bn3_b"], np.float32)
    m3, v3 = np.asarray(inputs["bn3_m"], np.float32), np.asarray(inputs["bn3_v"], np.float32)
    final_w = np.asarray(inputs["final_w"], np.float32)
    final_b = np.asarray(inputs["final_b"], np.float32)

    a1 = g1 / np.sqrt(v1 + EPS)                      # [300] > 0
    c1 = a1 * (conv_b - m1) + b1                     # [300]
    a2 = g2 / np.sqrt(v2 + EPS)                      # [300,100]
    c2 = b2 - a2 * m2 + a2 * fc1_b                   # [300,100]
    a3 = g3 / np.sqrt(v3 + EPS)                      # [300]
    c3 = a3 * (fc2_b - m3) + b3                      # [300]

    bf = ml_dtypes.bfloat16

    # conv weights [76, 384]: a1 folded in; cols >= 300 are zero pad
    wconv = np.zeros((CK, WCONV_COLS), np.float32)
    wconv[0:76, :N] = (conv_w * a1[:, None, None]).transpose(1, 2, 0).reshape(76, N)
    c1t = np.zeros((128, 3), np.float32)
    for ci in range(3):
        u0 = 128 * ci
        seg = c1[u0:min(u0 + 128, N)]
        c1t[0:len(seg), ci] = seg

    # fc1: lhsT [84, 128] per unit; rows 0..82 = a2*w1 (p-major),
    # row 83 = c2 (pairs with the ones row of pTall); h cols 100..127 zero
    w1aug = np.zeros((NPAD, LP + 1, HPAD), np.float32)
    w1aug[:N, :LP, :C1] = (fc1_w * a2[:, :, None]).transpose(0, 2, 1)
    w1aug[:N, LP, :C1] = c2

    # fc2: rhs [101, 1] per unit; rows 0..99 = a3*w2, row 100 = c3
    w2aug = np.zeros((NPAD, C1 + 1), np.float32)
    w2aug[:N, :C1] = fc2_w * a3[:, None]
    w2aug[:N, C1] = c3

    fwpad = np.zeros((NCLS, NPAD), np.float32)
    fwpad[:, :N] = final_w

    identity = np.eye(128, dtype=np.float32)

    in_maps = []
    for i in range(NCORES):
        us = slice(i * ULOC, (i + 1) * ULOC)
        w1c = w1aug[us].transpose(1, 0, 2).reshape(LP + 1, ULOC * HPAD)
        w2c = w2aug[us].T                                   # [101, 38]
        wp = np.zeros((128, WTOT), np.float32)
        wp[0:CK, W_CONV:W_CONV + WCONV_COLS] = wconv
        wp[0:LP + 1, W_W1:W_W1 + ULOC * HPAD] = w1c
        wp[0:C1 + 1, W_W2:W_W2 + ULOC] = w2c
        for cls in range(NCLS):
            wp[:, W_FW + cls * ULOC:W_FW + (cls + 1) * ULOC] = fwpad[cls, us][None, :]
        wp[:, W_ID:W_ID + 128] = identity
        in_maps.append({
            "xloc": np.ascontiguousarray(x[i * BLOC:(i + 1) * BLOC].transpose(1, 0, 2)).astype(bf),
            "wpack": wp.astype(bf),
            "c1p": c1t,
        })
    return in_maps, final_b


def kernel(**inputs):
    from concourse.bass_utils import run_bass_kernel_spmd

    if "nc" not in _CACHE:
        _CACHE["nc"] = _build_bass()
    nc = _CACHE["nc"]

    in_maps, final_b = _host_prep(inputs)
    res = run_bass_kernel_spmd(nc, in_maps, core_ids=list(range(NCORES)))
    out = np.zeros((B, NCLS), np.float32)
    for r in res.results:
        out += r["out_part"]
    out += final_b[None, :]
    return out
